# revision 3
# baseline (speedup 1.0000x reference)
"""Trainium2 Bass kernel for nn_Net_24275155157688.

Per batch element (64x64 adjacency x):
  tr_p = trace(x^p), s_p = sum(x^p) for p = 2..5
  feats(i,j) = [tr_{i+2}^(j+1)/4096^(i+j+1), s_{i+2}^(j+1)/4096^(i+j+2)]
  out = feats @ W.T                      (W: [2, 32])

Per pair of batches (stacked [x0; x1] on 128 partitions, quadrant matmuls):
  [T | u1] = x^T @ [I | 1]               (regular matmul; u1 = colsums(x))
  [P2|v1] = x@[x|1], [P3|v2] = x@[P2|v1], P4 = x@P3
  tr_{q+2} = sum(T o P_q)  (P_1 := x)    4 DVE fp16 products
  s2 = halfsum(v2); s3 = halfsum(v3), v3 = rowsums(P3) via TT-add tree
  s4 = halfsum(v4), v4 = rowsums(P4) via tree;  s5 = halfsum(u1 o v4)

Everything stays [128]-wide (narrow [2,..] DVE/ACT ops use 2/128 lanes and
are ~64x less parallel -- the v4..v7 mask-row designs all sank on this).
Row-sums use TT-add trees (2 fp16 els/cycle/lane) instead of tensor_reduce
(1 el/cycle/lane + ~0.4us fixed): the 4 trace products are written into ONE
[128, 2048] tile so one 3-stage tree reduces all 4 at once. Per group the
only narrow ops are one [2,64] mask-matmul (PE) + one [2,64] ACT copy.
GpSimd is never used (measured: GpSimd ops do not overlap with PE).

Data parallel across 8 NeuronCores: x[8192] -> 8 x [1024].
"""

import sys
import numpy as np

sys.path.insert(0, "/opt/trn_rl_repo")

import concourse.bass as bass
import concourse.bacc as bacc
import concourse.mybir as mybir
from concourse.tile import TileContext

F32 = mybir.dt.float32
F16 = mybir.dt.float16

NCORES = 8
B, N = 8192, 64
BPC = B // NCORES
GP = 8
GB = 2 * GP
HALVES = ((0, 64), (64, 128))


def make_consts():
    ident65 = np.zeros((128, 65), np.float16)
    for p in range(128):
        ident65[p, p % 64] = 1.0
    ident65[:, 64] = 1.0
    hm32 = np.zeros((128, 2), np.float32)
    hm32[0:64, 0] = 1.0
    hm32[64:128, 1] = 1.0
    hm16 = hm32.astype(np.float16)
    return ident65, hm32, hm16


def build_nc(bpc=BPC, repeat=1):
    ng = bpc // GB
    assert bpc % GB == 0

    nc = bacc.Bacc()
    x_d = nc.dram_tensor("x", [bpc, N, N], F32, kind="ExternalInput")
    id_d = nc.dram_tensor("ident65", [128, 65], F16, kind="ExternalInput")
    hm32_d = nc.dram_tensor("hm32", [128, 2], F32, kind="ExternalInput")
    hm16_d = nc.dram_tensor("hm16", [128, 2], F16, kind="ExternalInput")
    out_d = nc.dram_tensor("out", [2, 64 * ng], F32, kind="ExternalOutput")

    with TileContext(nc) as tc:
        with (
            tc.tile_pool(name="const", bufs=1) as constp,
            tc.tile_pool(name="x32", bufs=4) as x32p,
            tc.tile_pool(name="xh", bufs=1) as xhp,
            tc.tile_pool(name="wtsb", bufs=5) as wtp,
            tc.tile_pool(name="p2sb", bufs=4) as p2p,
            tc.tile_pool(name="c34", bufs=4) as c34p,
            tc.tile_pool(name="prods", bufs=3) as prodp,
            tc.tile_pool(name="t1", bufs=3) as t1p,
            tc.tile_pool(name="t2", bufs=2) as t2p,
            tc.tile_pool(name="coll", bufs=4) as collp,
            tc.tile_pool(name="call", bufs=1) as callp,
            tc.tile_pool(name="wtps", bufs=1, space="PSUM") as wtpsp,
            tc.tile_pool(name="p2ps", bufs=1, space="PSUM") as p2psp,
            tc.tile_pool(name="p3ps", bufs=1, space="PSUM") as p3psp,
            tc.tile_pool(name="p4ps", bufs=1, space="PSUM") as p4psp,
            tc.tile_pool(name="ops", bufs=1, space="PSUM") as opsp,
        ):
            ident65 = constp.tile([128, 65], F16)
            nc.sync.dma_start(out=ident65[:], in_=id_d[:])
            hm32 = constp.tile([128, 2], F32)
            nc.sync.dma_start(out=hm32[:], in_=hm32_d[:])
            hm16 = constp.tile([128, 2], F16)
            nc.sync.dma_start(out=hm16[:], in_=hm16_d[:])

            call = callp.tile([2, 64 * ng], F32)

            # xh buffers: [x_j | 1] 65-wide blocks; ones cols set once
            xh_bufs = []
            for i in range(5):
                t = xhp.tile([128, GP * 65], F16, tag=f"xh{i}")
                nc.vector.memset(
                    t[:].rearrange("p (j c) -> p j c", c=65)[:, :, 64:65], 1.0
                )
                xh_bufs.append(t)

            rep_ctx = tc.For_i(0, repeat, 1) if repeat > 1 else None
            if rep_ctx is not None:
                rep_ctx.__enter__()

            st = {}

            def b65(t):
                return t[:].rearrange("p (j c) -> p j c", c=65)

            def m64(t):
                return b65(t)[:, :, 0:64]

            def s128(ps):
                return ps[:].rearrange("p (j c) -> p j c", c=128)[:, :, 0:65]

            def stage_load(g):
                x32 = x32p.tile([128, GP * 64], F32, tag="x32")
                src = (
                    x_d.rearrange("b r c -> (b r) c")[g * 1024 : (g + 1) * 1024]
                    .rearrange("(pr p) c -> p pr c", pr=GP)
                )
                nc.sync.dma_start(
                    out=x32[:].rearrange("p (k c) -> p k c", c=64), in_=src
                )
                xh = xh_bufs[g % 5]
                nc.scalar.copy(
                    out=m64(xh),
                    in_=x32[:].rearrange("p (j c) -> p j c", c=64),
                )
                wtps = wtpsp.tile([128, GP * 128], F32, tag="wtps")
                for j in range(GP):
                    for lo, hi in HALVES:
                        nc.tensor.matmul(
                            wtps[lo:hi, 128 * j : 128 * j + 65],
                            xh[lo:hi, 65 * j : 65 * j + 64],
                            ident65[lo:hi, :],
                        )
                wtsb = wtp.tile([128, GP * 65], F16, tag="wtsb")
                nc.scalar.copy(out=b65(wtsb), in_=s128(wtps))
                st[g] = {"xh": xh, "wt": wtsb}

            def chain65(g, key, rhs_key, psp, sbp_, tag):
                # [P_k | v] = x @ [P_{k-1} | v_prev]   (65-col blocks)
                d = st[g]
                ps = psp.tile([128, GP * 128], F32, tag=tag + "ps")
                for j in range(GP):
                    for lo, hi in HALVES:
                        nc.tensor.matmul(
                            ps[lo:hi, 128 * j : 128 * j + 65],
                            d["wt"][lo:hi, 65 * j : 65 * j + 64],
                            d[rhs_key][lo:hi, 65 * j : 65 * j + 65],
                        )
                if tag == "p3sb":
                    # P3 goes into section 0 of the shared c34 tile
                    c34 = c34p.tile([128, 2 * GP * 65], F16, tag="c34")
                    nc.scalar.copy(
                        out=c34[:, 0 : GP * 65].rearrange(
                            "p (j c) -> p j c", c=65
                        ),
                        in_=s128(ps),
                    )
                    d["c34"] = c34
                    d[key] = c34[:, 0 : GP * 65]
                else:
                    sb = sbp_.tile([128, GP * 65], F16, tag=tag)
                    nc.scalar.copy(out=b65(sb), in_=s128(ps))
                    d[key] = sb

            def stage_p4(g):
                # P4 = x @ P3 -> section 1 of c34 (65-stride, col 64 junk)
                d = st[g]
                p4ps = p4psp.tile([128, GP * 64], F32, tag="p4ps")
                for j in range(GP):
                    cs = slice(j * 64, j * 64 + 64)
                    for lo, hi in HALVES:
                        nc.tensor.matmul(
                            p4ps[lo:hi, cs], d["wt"][lo:hi, 65 * j : 65 * j + 64],
                            d["p3"][lo:hi, 65 * j : 65 * j + 64],
                        )
                c34 = d["c34"]
                nc.scalar.copy(
                    out=c34[:, GP * 65 : 2 * GP * 65].rearrange(
                        "p (j c) -> p j c", c=65
                    )[:, :, 0:64],
                    in_=p4ps[:].rearrange("p (j c) -> p j c", c=64),
                )
                d["p4"] = c34[:, GP * 65 : 2 * GP * 65]

            def stage_red(g):
                d = st[g]
                wt = d["wt"]
                # 4 trace products into one [128, 2048] tile
                prods = prodp.tile([128, 4 * GP * 64], F16, tag="prods")
                for q, (a, b) in enumerate((
                    (m64(d["xh"]), m64(wt)),
                    (m64(wt), m64(d["p2"])),
                    (m64(wt), m64(d["p3"])),
                    (m64(wt), d["p4"].rearrange("p (j c) -> p j c", c=65)[:, :, 0:64]),
                )):
                    nc.vector.tensor_mul(
                        prods[:, 512 * q : 512 * q + 512]
                        .rearrange("p (j c) -> p j c", c=64),
                        a, b,
                    )
                C = collp.tile([128, 56], F32, tag="coll")
                # 3-stage TT-add tree over all 4 products at once -> C[:, 0:32]
                t1 = t1p.tile([128, 4 * GP * 32], F16, tag="t1")
                pv = prods[:].rearrange("p (m c) -> p m c", c=64)
                nc.vector.tensor_add(
                    t1[:].rearrange("p (m c) -> p m c", c=32),
                    pv[:, :, 0:32], pv[:, :, 32:64],
                )
                t2 = t2p.tile([128, 4 * GP * 16], F32, tag="t2")
                tv1 = t1[:].rearrange("p (m c) -> p m c", c=32)
                nc.vector.tensor_add(
                    t2[:].rearrange("p (m c) -> p m c", c=16),
                    tv1[:, :, 0:16], tv1[:, :, 16:32],
                )
                tv2 = t2[:].rearrange("p (m c) -> p m c", c=16)
                # final: 16 -> 8+8 -> need full collapse to 1 per (q, j):
                # two more halvings on fp32
                t3 = t2p.tile([128, 4 * GP * 8], F32, tag="t3")
                nc.vector.tensor_add(
                    t3[:].rearrange("p (m c) -> p m c", c=8),
                    tv2[:, :, 0:8], tv2[:, :, 8:16],
                )
                tv3 = t3[:].rearrange("p (m c) -> p m c", c=8)
                t4 = t2p.tile([128, 4 * GP * 4], F32, tag="t4")
                nc.vector.tensor_add(
                    t4[:].rearrange("p (m c) -> p m c", c=4),
                    tv3[:, :, 0:4], tv3[:, :, 4:8],
                )
                tv4 = t4[:].rearrange("p (m c) -> p m c", c=4)
                t5 = t2p.tile([128, 4 * GP * 2], F32, tag="t5")
                nc.vector.tensor_add(
                    t5[:].rearrange("p (m c) -> p m c", c=2),
                    tv4[:, :, 0:2], tv4[:, :, 2:4],
                )
                tv5 = t5[:].rearrange("p (m c) -> p m c", c=2)
                nc.vector.tensor_add(
                    C[:, 0:32].rearrange("p (m c) -> p m c", c=1),
                    tv5[:, :, 0:1], tv5[:, :, 1:2],
                )
                # s2 = halfsum(v2): v2 cols masked directly in stage_fin
                # v3, v4 = rowsums(P3), rowsums(P4): ONE X-reduce over the
                # shared c34 tile (both sections, 65-stride, cols 0:64)
                nc.vector.tensor_reduce(
                    C[:, 32:48],
                    d["c34"][:].rearrange("p (m c) -> p m c", c=65)[:, :, 0:64],
                    axis=mybir.AxisListType.X,
                    op=mybir.AluOpType.add,
                )
                # s5 = halfsum(u1 o v4)
                nc.vector.tensor_mul(
                    C[:, 48:56].rearrange("p (j c) -> p j c", c=1),
                    b65(wt)[:, :, 64:65],
                    C[:, 40:48].rearrange("p (j c) -> p j c", c=1),
                )
                d["C"] = C

            def stage_fin(g):
                d = st.pop(g)
                ops = opsp.tile([2, 64], F32, tag="ops")
                nc.tensor.matmul(ops[:, 0:56], hm32[:], d["C"][:])
                # s2: half-sums of the v2 columns (65-stride col 64 of c34 p3)
                nc.tensor.matmul(
                    ops[:, 56:64], hm16[:],
                    d["c34"][:].rearrange("p (j c) -> p j c", c=65)[
                        :, 0:GP, 64:65
                    ].rearrange("p j c -> p (j c)"),
                )
                nc.scalar.copy(out=call[:, 64 * g : 64 * g + 64], in_=ops[:])

            for it in range(ng + 5):
                if 0 <= it - 4 < ng:
                    stage_fin(it - 4)
                if it < ng:
                    stage_load(it)
                if 0 <= it - 1 < ng:
                    chain65(it - 1, "p2", "xh", p2psp, p2p, "p2sb")
                if 0 <= it - 2 < ng:
                    stage_p4(it - 2)
                if 0 <= it - 1 < ng:
                    chain65(it - 1, "p3", "p2", p3psp, None, "p3sb")
                if 0 <= it - 3 < ng:
                    stage_red(it - 3)

            nc.sync.dma_start(out=out_d[:], in_=call[:])

            if rep_ctx is not None:
                rep_ctx.__exit__(None, None, None)

    nc.compile()
    return nc


# ---------------------------------------------------------------------------
# host side
# ---------------------------------------------------------------------------

_NC_CACHE = {}


def _get_nc(bpc, repeat=1):
    key = (bpc, repeat)
    if key not in _NC_CACHE:
        _NC_CACHE[key] = build_nc(bpc, repeat)
    return _NC_CACHE[key]


def _host_finish(out, W, bpc):
    """out [2, 64*ng] -> [bpc, 2].

    cols per group: 0:32 = tr2..tr5 x 8 pairs | 32:40 s3 | 40:48 s4 |
    48:56 s5 | 56:64 s2
    """
    ng = bpc // GB
    numel = float(N * N)
    vals = out.reshape(2, ng, 8, GP).astype(np.float64)  # [h, g, slot, j]
    tr = np.empty((4, bpc), np.float64)
    s = np.empty((4, bpc), np.float64)
    gg, jj, hh = np.meshgrid(
        np.arange(ng), np.arange(GP), np.arange(2), indexing="ij"
    )
    bidx = (GB * gg + 2 * jj + hh).ravel()
    smap = {0: 7, 1: 4, 2: 5, 3: 6}  # s2<-slot7, s3<-slot4, s4<-slot5, s5<-slot6
    for k in range(4):
        tr[k, bidx] = vals[:, :, k, :].transpose(1, 2, 0).ravel()
        s[k, bidx] = vals[:, :, smap[k], :].transpose(1, 2, 0).ravel()
    feats = np.empty((bpc, 32), np.float64)
    for i in range(4):
        gsc = tr[i] / numel
        hsc = s[i] / numel
        for j in range(4):
            feats[:, 4 * i + j] = gsc ** (j + 1) / numel**i
            feats[:, 16 + 4 * i + j] = hsc ** (j + 1) / numel ** (i + 1)
    return feats @ W.astype(np.float64).T


def make_in_maps(x):
    ident65, hm32, hm16 = make_consts()
    return [
        {
            "x": np.ascontiguousarray(x[c * BPC : (c + 1) * BPC]),
            "ident65": ident65,
            "hm32": hm32,
            "hm16": hm16,
        }
        for c in range(NCORES)
    ]


def _run(x, W, trace=False):
    from concourse.bass_utils import run_bass_kernel_spmd

    nc = _get_nc(BPC)
    in_maps = make_in_maps(x)
    r = run_bass_kernel_spmd(nc, in_maps, list(range(NCORES)), trace=trace)
    res = r.results
    out = np.empty((B, 2), np.float32)
    for c in range(NCORES):
        out[c * BPC : (c + 1) * BPC] = _host_finish(
            res[c]["out"], W, BPC
        ).astype(np.float32)
    return out, r


def kernel(x, W):
    return _run(x, W)[0]


def run_traced(x, W):
    out, r = _run(x, W, trace=True)
    return r.exec_time_ns
